# revision 35
# baseline (speedup 1.0000x reference)
"""Trainium2 Bass kernel for a pre-norm transformer block (B=8, N=1024, C=768).

Data-parallel over batch: each of the 8 NeuronCores runs the full block for
one batch element. Activations are feature-major ([feat, tok]) so matmuls
contract over the partition dim with no on-device transposes.

vs the f32r baseline:
  - bf16 matmul operands end-to-end (PSUM accumulation stays f32); rel err
    ~1e-3, well under the 2e-2 gate. Halves DMA and enables DVE 2x modes.
  - LayerNorm gain/bias folded into the downstream weights/biases on the
    host, so normalize is 2 DVE ops (sub, mul) per tile; squares for the
    sumsq stat run on the ACT engine.
  - All linear-layer biases are applied as ones-row matmuls folded into the
    PSUM accumulation groups (one extra 512/1024-row stream per output
    tile), so no elementwise bias pass exists; PSUM->SBUF moves run on ACT.
  - Wide [128,1024] 2-bank PSUM tiles: one exp per head-PAIR per key-chunk,
    one gelu per fc1 row-pair, halving ACT instruction count.
  - Attention is software-pipelined: scores(kc) emit before PV(kc-1), and
    the next head-pair's q/k projection matmuls are drip-fed between key
    chunks to fill PE idle slots while ACT works on exp.
  - fc1/fc2 weights stream once per rep (full-width token processing), not
    once per token-half.
"""

import os
import sys

import numpy as np

for _p in ("/opt/trn_rl_repo", "/root/.axon_site/_ro/trn_rl_repo"):
    if os.path.isdir(_p) and _p not in sys.path:
        sys.path.append(_p)

import concourse.bass as bass  # noqa: E402
import concourse.tile as tile  # noqa: E402
from concourse import bacc, mybir  # noqa: E402
from concourse.bass_utils import run_bass_kernel_spmd  # noqa: E402

F32 = mybir.dt.float32
BF16 = mybir.dt.bfloat16
AF = mybir.ActivationFunctionType
OP = mybir.AluOpType

P = 128
D = 768
KD = D // P          # 6 feature chunks over the 768 contraction dim
NTOK = 1024
F = 512              # token-half width (matmul free dim)
NHALF = NTOK // F    # 2
H = 12
DK = 64
DFF = 3072
MF1 = DFF // P       # 24
TC = NTOK // P       # 8 key/token chunks
EPS = 1e-5
N_CORES = 8


def build_program(reps=1):
    stage = int(os.environ.get("KERNEL_STAGE", "5"))
    nc = bacc.Bacc(
        "TRN2", target_bir_lowering=False, debug=False, num_devices=N_CORES
    )

    din = lambda name, shape, dt=BF16: nc.dram_tensor(
        name, shape, dt, kind="ExternalInput"
    ).ap()
    xt = din("xt", [P, KD, NTOK])
    onesb = din("onesb", [P, 1])
    wqk = din("wqk", [2 * KD, P, KD, P])
    wv = din("wv", [P, KD, KD, P])
    bqkvr = din("bqkvr", [1, 3 * D])
    wproj = din("wproj", [P, KD, KD, P])
    bprojr = din("bprojr", [1, D])
    wfc1 = din("wfc1", [MF1, P, KD, P])
    bfc1 = din("bfc1", [P, MF1], F32)
    wfc2 = din("wfc2", [KD, P, MF1, P])
    bfc2r = din("bfc2r", [1, D])
    yt = nc.dram_tensor("yt", [P, KD, NTOK], F32, kind="ExternalOutput").ap()

    with tile.TileContext(nc) as tc:
        # PSUM pools are all per-phase (8-bank budget): qkv spp(3 wide),
        # attention spA(2 wide)+opsw(2 wide), MLP mlpw(4 wide).
        # ---- constant/global SBUF pools ----
        const = tc.alloc_tile_pool(name="const", bufs=1)
        stat = tc.alloc_tile_pool(name="stat", bufs=5)
        bc1 = tc.alloc_tile_pool(name="bc1", bufs=4)     # [1,F] bf16 casts
        bcP = tc.alloc_tile_pool(name="bcP", bufs=4)     # [P,F] bf16 bcasts
        bcR = tc.alloc_tile_pool(name="bcR", bufs=4)     # [DK,F] f32 bcasts
        sqp = tc.alloc_tile_pool(name="sqp", bufs=6)
        tmp = tc.alloc_tile_pool(name="tmp", bufs=3)
        ptp = tc.alloc_tile_pool(name="ptp", bufs=4)
        outp = tc.alloc_tile_pool(name="outp", bufs=2)
        wstream = tc.alloc_tile_pool(name="wstream", bufs=6)
        f1s = tc.alloc_tile_pool(name="f1s", bufs=8)
        f2s = tc.alloc_tile_pool(name="f2s", bufs=2)

        eps_sb = const.tile([1, 1], F32)
        nc.vector.memset(eps_sb, EPS)
        onesrow = const.tile([1, NTOK], BF16)
        nc.vector.memset(onesrow, 1.0)
        onesb_sb = const.tile([P, 1], BF16, name="onesb_sb")
        nc.sync.dma_start(out=onesb_sb[:], in_=onesb[:])

        def load_const(ap_dram, shape=None, dt=None):
            t = const.tile(
                shape or list(ap_dram.shape), dt or ap_dram.dtype,
                name=ap_dram.name + "_sb",
            )
            nc.sync.dma_start(out=t[:], in_=ap_dram[:])
            return t

        bqr_sb = load_const(bqkvr)
        bpr_sb = load_const(bprojr)
        bfc1_sb = load_const(bfc1)
        bfc2r_sb = load_const(bfc2r)
        wv_sb = load_const(wv)
        wproj_sb = load_const(wproj)

        # ---- layernorm: stats via ones-matmuls (one wide PSUM tile:
        # sum in bank 0, sumsq in bank 1), 2-op normalize ----
        def emit_ln_half(src, dst, half, pool):
            cols = slice(half * F, (half + 1) * F)
            st_ps = pool.tile([P, 2 * F], F32, tag="wd", name="st_ps")
            # Hoist the first squares so ACT overlaps the PE sum pass.
            sqs = []
            for kk in range(3):
                sq = sqp.tile([P, F], BF16, tag="sq", name="sq")
                nc.scalar.activation(sq[:], src[:, kk, cols], AF.Square)
                sqs.append(sq)
            for kk in range(KD):
                nc.tensor.matmul(
                    st_ps[0:1, 0:F], onesb_sb[:], src[:, kk, cols],
                    start=(kk == 0), stop=(kk == KD - 1),
                )
            for kk in range(KD):
                if kk >= 3:
                    sq = sqp.tile([P, F], BF16, tag="sq", name="sq")
                    nc.scalar.activation(sq[:], src[:, kk, cols], AF.Square)
                    sqs.append(sq)
                nc.tensor.matmul(
                    st_ps[0:1, F : 2 * F], onesb_sb[:], sqs[kk][:],
                    start=(kk == 0), stop=(kk == KD - 1),
                )
            mu = stat.tile([1, F], F32, tag="st", name="mu")
            nc.vector.tensor_scalar_mul(mu[:], st_ps[0:1, 0:F], 1.0 / D)
            e2 = stat.tile([1, F], F32, tag="st", name="e2")
            nc.vector.tensor_scalar_mul(e2[:], st_ps[0:1, F : 2 * F], 1.0 / D)
            m2 = stat.tile([1, F], F32, tag="st", name="m2")
            nc.vector.tensor_mul(m2[:], mu[:], mu[:])
            nc.vector.tensor_tensor(e2[:], e2[:], m2[:], OP.subtract)
            sd = stat.tile([1, F], F32, tag="st", name="sd")
            nc.scalar.activation(sd[:], e2[:], AF.Sqrt, bias=eps_sb[0:1])
            rs = stat.tile([1, F], F32, tag="st", name="rs")
            nc.vector.reciprocal(rs[:], sd[:])
            mu16 = bc1.tile([1, F], BF16, tag="b1", name="mu16")
            nc.vector.tensor_copy(out=mu16[:], in_=mu[:])
            rs16 = bc1.tile([1, F], BF16, tag="b1", name="rs16")
            nc.vector.tensor_copy(out=rs16[:], in_=rs[:])
            mu_b = bcP.tile([P, F], BF16, tag="bP", name="mu_b")
            nc.gpsimd.partition_broadcast(mu_b[:], mu16[:])
            rs_b = bcP.tile([P, F], BF16, tag="bP", name="rs_b")
            nc.gpsimd.partition_broadcast(rs_b[:], rs16[:])
            for kk in range(KD):
                t1 = tmp.tile([P, F], BF16, tag="t", name="t1")
                nc.vector.tensor_tensor(
                    t1[:], src[:, kk, cols], mu_b[:], OP.subtract
                )
                nc.vector.tensor_mul(dst[:, kk, cols], t1[:], rs_b[:])

        for _rep in range(reps):
            spp = tc.alloc_tile_pool(name="spp", bufs=3, space="PSUM")

            xt_pool = tc.alloc_tile_pool(name="xt", bufs=1)
            attn_pool = tc.alloc_tile_pool(name="attn", bufs=1)
            qk_pool = tc.alloc_tile_pool(name="qk", bufs=12)
            vaug_pool = tc.alloc_tile_pool(name="vaug", bufs=1)
            h_pool = tc.alloc_tile_pool(name="h", bufs=1)

            xt_sb = xt_pool.tile([P, KD, NTOK], BF16, name="xt_sb")
            for half in range(NHALF):
                for kk in range(KD):
                    cols = slice(half * F, (half + 1) * F)
                    nc.sync.dma_start(out=xt_sb[:, kk, cols], in_=xt[:, kk, cols])

            def dump_and_skip(pools):
                for m in range(KD):
                    yo = outp.tile([P, 2 * F], F32, tag="yo", name="yo")
                    nc.vector.memset(yo[:], 0.0)
                    nc.sync.dma_start(out=yt[:, m, :], in_=yo[:])
                for p in pools:
                    p.release()

            hT = h_pool.tile([P, KD, NTOK], BF16, name="hT")
            emit_ln_half(xt_sb, hT, 0, spp)
            emit_ln_half(xt_sb, hT, 1, spp)

            if stage <= 1:
                dump_and_skip([h_pool, vaug_pool, qk_pool, attn_pool,
                               xt_pool, spp])
                continue

            attnT = attn_pool.tile([P, KD, NTOK], BF16, name="attnT")
            v_aug = vaug_pool.tile([P, TC, H, DK + 1], BF16, name="v_aug")
            nc.vector.memset(v_aug[:, :, :, DK : DK + 1], 1.0)

            qk_tiles = {}

            # ---- q/k chunk: 12 matmuls + 2 bias rows -> ACT move to bf16 ----
            def qk_chunk_ops(m):
                wt = wstream.tile([P, KD, P], BF16, tag="w", name="wt")
                nc.sync.dma_start(out=wt[:], in_=wqk[m])
                qkt = qk_pool.tile([P, NTOK], BF16, tag="qkt", name="qkt")
                qk_tiles[m] = qkt
                holder = {}

                def group(half):
                    if half == 0:
                        holder["ps"] = spp.tile(
                            [P, 2 * F], F32, tag="wd", name="qkps"
                        )
                    c0 = half * F
                    for kk in range(KD):
                        nc.tensor.matmul(
                            holder["ps"][:, c0 : c0 + F],
                            wt[:, kk, :], hT[:, kk, c0 : c0 + F],
                            start=(kk == 0), stop=False,
                        )
                    nc.tensor.matmul(
                        holder["ps"][:, c0 : c0 + F],
                        bqr_sb[0:1, m * P : (m + 1) * P],
                        onesrow[0:1, 0:F],
                        start=False, stop=True,
                    )

                def move():
                    nc.scalar.activation(qkt[:, :], holder["ps"][:, :], AF.Copy)

                return group, move

            def emit_qk_full(m):
                g, mv = qk_chunk_ops(m)
                g(0)
                g(1)
                mv()

            # ---- v chunk (token-major) ----
            def emit_v(t):
                trange = slice(t * P, (t + 1) * P)
                ps = spp.tile([P, 2 * F], F32, tag="wd", name="vps")
                for c0, w in ((0, 512), (512, 256)):
                    for kk in range(KD):
                        nc.tensor.matmul(
                            ps[:, c0 : c0 + w],
                            hT[:, kk, trange],
                            wv_sb[:, kk, c0 // P : (c0 + w) // P, :],
                            start=(kk == 0), stop=False,
                        )
                    nc.tensor.matmul(
                        ps[:, c0 : c0 + w],
                        onesrow[0:1, 0:P],
                        bqr_sb[0:1, 2 * D + c0 : 2 * D + c0 + w],
                        start=False, stop=True,
                    )
                nc.scalar.activation(
                    v_aug[:, t, :, 0:DK],
                    ps[:, 0:D].rearrange("p (h d) -> p h d", d=DK),
                    AF.Copy,
                )

            # qkv phase: first chunks split-emitted (h0 groups first) so the
            # PE never waits on LN1-h1 normalize; spp ring-3 rotation keeps
            # three tiles in flight.
            g_q0, mv_q0 = qk_chunk_ops(0)
            g_k0, mv_k0 = qk_chunk_ops(6)
            g_q1, mv_q1 = qk_chunk_ops(1)
            g_k1, mv_k1 = qk_chunk_ops(7)
            g_q0(0)
            g_k0(0)
            g_q1(0)
            g_q0(1); mv_q0()
            g_k1(0)
            g_k0(1); mv_k0()
            emit_v(0)
            g_q1(1); mv_q1()
            emit_v(1)
            g_k1(1); mv_k1()
            for t in range(2, TC):
                emit_v(t)
            for m in (2, 8, 3, 9, 4, 10, 5, 11):
                emit_qk_full(m)

            if stage <= 2:
                dump_and_skip([h_pool, vaug_pool, qk_pool, attn_pool,
                               xt_pool, spp])
                continue

            # ---- attention: double-buffered score tiles AND PV accumulators
            # so PE never couples to the ACT exp backlog ----
            spp.release()
            spA = tc.alloc_tile_pool(name="spA", bufs=2, space="PSUM")
            opsw = tc.alloc_tile_pool(name="opsw", bufs=2, space="PSUM")
            pranges = (slice(0, DK), slice(DK, P))

            # One global software pipeline over all (pair, half, kc) units:
            # S+exp for unit i, then PV for unit i-1 — no per-half drain.
            state = {}  # (j, half) -> dict(o_ps=..., pts={kc: pt})

            def emit_s_exp(j, half, kc):
                st = state.setdefault((j, half), {"pts": {}})
                if kc == 0:
                    st["o_ps"] = opsw.tile([P, 2 * F], F32, tag="ow",
                                           name="o_ps")
                q_t = qk_tiles[j]
                k_t = qk_tiles[KD + j]
                cols = slice(half * F, (half + 1) * F)
                sp = spA.tile([P, 2 * F], F32, tag="sp", name="sp")
                for hi in (0, 1):
                    pr = pranges[hi]
                    nc.tensor.matmul(
                        sp[:, hi * F : (hi + 1) * F],
                        k_t[pr, kc * P : (kc + 1) * P],
                        q_t[pr, cols],
                        start=True, stop=True,
                    )
                pt = ptp.tile([P, 2 * F], BF16, tag="pt", name="pt")
                nc.scalar.activation(
                    pt[:], sp[:], AF.Exp, scale=float(DK) ** -0.5
                )
                st["pts"][kc] = pt

            def emit_pv(j, half, kc):
                st = state[(j, half)]
                o_ps = st["o_ps"]
                pt = st["pts"].pop(kc)
                for hi in (0, 1):
                    nc.tensor.matmul(
                        o_ps[0 : DK + 1, hi * F : (hi + 1) * F],
                        v_aug[:, kc, 2 * j + hi, :],
                        pt[:, hi * F : (hi + 1) * F],
                        start=(kc == 0), stop=(kc == TC - 1),
                    )
                if kc == TC - 1:
                    cols = slice(half * F, (half + 1) * F)
                    for hi in (0, 1):
                        rec = stat.tile([1, F], F32, tag="st", name="rec")
                        nc.vector.reciprocal(
                            rec[:], o_ps[DK : DK + 1, hi * F : (hi + 1) * F]
                        )
                        rec_b = bcR.tile([DK, F], F32, tag="bR", name="rec_b")
                        nc.gpsimd.partition_broadcast(rec_b[:], rec[:])
                        nc.vector.tensor_mul(
                            attnT[pranges[hi], j, cols],
                            o_ps[0:DK, hi * F : (hi + 1) * F], rec_b[:],
                        )

            units = [(j, half, kc)
                     for j in range(KD) for half in range(NHALF)
                     for kc in range(TC)]
            SKEW = 1  # PE stays 1 S+exp unit ahead of the PV stream
            for i, u in enumerate(units):
                emit_s_exp(*u)
                if i >= SKEW:
                    emit_pv(*units[i - SKEW])
            for u in units[-SKEW:]:
                emit_pv(*u)

            if stage <= 3:
                dump_and_skip([h_pool, vaug_pool, qk_pool, attn_pool,
                               xt_pool, opsw, spA])
                continue

            h_pool.release()
            vaug_pool.release()
            qk_pool.release()
            opsw.release()
            spA.release()

            # ---- proj + residual -> x2T ----
            mlpw = tc.alloc_tile_pool(name="mlpw", bufs=4, space="PSUM")
            x2_pool = tc.alloc_tile_pool(name="x2", bufs=1, side="right")
            h2_pool = tc.alloc_tile_pool(name="h2", bufs=1, side="right")
            g_pool = tc.alloc_tile_pool(name="g", bufs=1, side="right")
            x2T = x2_pool.tile([P, KD, NTOK], BF16, name="x2T")
            h2T = h2_pool.tile([P, KD, NTOK], BF16, name="h2T")
            gT = g_pool.tile([P, MF1, NTOK], BF16, name="gT")

            for m in range(KD):
                ps = mlpw.tile([P, 2 * F], F32, tag="wd", name="prps")
                for half in range(NHALF):
                    c0 = half * F
                    for kk in range(KD):
                        nc.tensor.matmul(
                            ps[:, c0 : c0 + F],
                            wproj_sb[:, kk, m, :],
                            attnT[:, kk, c0 : c0 + F],
                            start=(kk == 0), stop=False,
                        )
                    nc.tensor.matmul(
                        ps[:, c0 : c0 + F],
                        bpr_sb[0:1, m * P : (m + 1) * P],
                        onesrow[0:1, 0:F],
                        start=False, stop=True,
                    )
                nc.vector.tensor_tensor(
                    x2T[:, m, :], ps[:, :], xt_sb[:, m, :], OP.add
                )

            # ---- LN2 ----
            emit_ln_half(x2T, h2T, 0, mlpw)
            emit_ln_half(x2T, h2T, 1, mlpw)

            if stage <= 4:
                dump_and_skip([g_pool, h2_pool, x2_pool, attn_pool,
                               xt_pool, mlpw])
                continue

            # ---- fc1: m0..5 half-at-a-time (hides LN2-h1 latency), rest
            # full-width; two m's share one wide PSUM tile ----
            f1_tiles = []
            for m in range(KD):
                wt = f1s.tile([P, KD, P], BF16, tag="f1", name="f1w")
                nc.sync.dma_start(out=wt[:], in_=wfc1[m])
                f1_tiles.append(wt)
            for c0 in (0, F):
                for mp in range(KD // 2):
                    ps = mlpw.tile([P, 2 * F], F32, tag="wd", name="f1ps")
                    for sub in (0, 1):
                        m = 2 * mp + sub
                        for kk in range(KD):
                            nc.tensor.matmul(
                                ps[:, sub * F : (sub + 1) * F],
                                f1_tiles[m][:, kk, :], h2T[:, kk, c0 : c0 + F],
                                start=(kk == 0), stop=(kk == KD - 1),
                            )
                    for sub in (0, 1):
                        m = 2 * mp + sub
                        nc.scalar.activation(
                            gT[:, m, c0 : c0 + F],
                            ps[:, sub * F : (sub + 1) * F],
                            AF.Gelu, bias=bfc1_sb[:, m : m + 1],
                        )
            for m in range(KD, MF1):
                wt = f1s.tile([P, KD, P], BF16, tag="f1", name="f1w")
                nc.sync.dma_start(out=wt[:], in_=wfc1[m])
                ps = mlpw.tile([P, 2 * F], F32, tag="wd", name="f1wd")
                for half in range(NHALF):
                    c0 = half * F
                    for kk in range(KD):
                        nc.tensor.matmul(
                            ps[:, c0 : c0 + F],
                            wt[:, kk, :], h2T[:, kk, c0 : c0 + F],
                            start=(kk == 0), stop=(kk == KD - 1),
                        )
                nc.scalar.activation(
                    gT[:, m, :], ps[:], AF.Gelu, bias=bfc1_sb[:, m : m + 1]
                )

            # ---- fc2 + residual -> out ----
            for m in range(KD):
                w2 = f2s.tile([P, MF1, P], BF16, tag="f2", name="f2w")
                nc.sync.dma_start(out=w2[:], in_=wfc2[m])
                ps = mlpw.tile([P, 2 * F], F32, tag="wd", name="f2wd")
                for half in range(NHALF):
                    c0 = half * F
                    for kk in range(MF1):
                        nc.tensor.matmul(
                            ps[:, c0 : c0 + F],
                            w2[:, kk, :], gT[:, kk, c0 : c0 + F],
                            start=(kk == 0), stop=False,
                        )
                    nc.tensor.matmul(
                        ps[:, c0 : c0 + F],
                        bfc2r_sb[0:1, m * P : (m + 1) * P],
                        onesrow[0:1, 0:F],
                        start=False, stop=True,
                    )
                yo = outp.tile([P, 2 * F], F32, tag="yo", name="yo")
                nc.vector.tensor_tensor(yo[:], ps[:], x2T[:, m, :], OP.add)
                nc.sync.dma_start(out=yt[:, m, :], in_=yo[:])

            g_pool.release()
            h2_pool.release()
            x2_pool.release()
            attn_pool.release()
            xt_pool.release()
            mlpw.release()

        f2s.release()
        f1s.release()
        wstream.release()
        outp.release()
        ptp.release()
        tmp.release()
        sqp.release()
        bcR.release()
        bcP.release()
        bc1.release()
        stat.release()
        const.release()

    nc.compile()
    return nc


def _retile_w(w_t, mtiles):
    """[out, in] weight -> [mtiles, P, in//P, P]: t[m,p,kk,o] = w[m*P+o, kk*P+p]."""
    out_dim, in_dim = w_t.shape
    a = w_t.reshape(mtiles, P, in_dim // P, P).transpose(0, 3, 2, 1)
    return np.ascontiguousarray(a)


def _rhs_tile(w_t):
    """[KD*P, in] weight -> [P, in//P, KD, P]: t[p,kk,m,o] = w[m*P+o, kk*P+p]."""
    a = w_t.reshape(KD, P, w_t.shape[1] // P, P).transpose(3, 2, 0, 1)
    return np.ascontiguousarray(a)


_NC_CACHE = {}


def _get_nc():
    if "nc" not in _NC_CACHE:
        _NC_CACHE["nc"] = build_program()
    return _NC_CACHE["nc"]


def prep_inputs(x, ln1_w, ln1_b, qkv_w, qkv_b, proj_w, proj_b,
                ln2_w, ln2_b, fc1_w, fc1_b, fc2_w, fc2_b):
    import ml_dtypes

    bf16 = np.dtype(ml_dtypes.bfloat16)
    f32 = lambda a: np.asarray(a, dtype=np.float32)
    x = f32(x)
    qkv_w, qkv_b = f32(qkv_w), f32(qkv_b)
    fc1_w, fc1_b = f32(fc1_w), f32(fc1_b)

    # fold LN gain/bias into the consuming layer
    wqkv_eff = qkv_w * f32(ln1_w)[None, :]
    bqkv_eff = qkv_b + qkv_w @ f32(ln1_b)
    wfc1_eff = fc1_w * f32(ln2_w)[None, :]
    bfc1_eff = fc1_b + fc1_w @ f32(ln2_b)

    shared = {
        "onesb": np.ones((P, 1), dtype=bf16),
        "wqk": _retile_w(wqkv_eff[: 2 * D], 2 * KD).astype(bf16),
        "wv": _rhs_tile(wqkv_eff[2 * D :]).astype(bf16),
        "bqkvr": np.ascontiguousarray(bqkv_eff[None, :]).astype(bf16),
        "wproj": _rhs_tile(f32(proj_w)).astype(bf16),
        "bprojr": np.ascontiguousarray(f32(proj_b)[None, :]).astype(bf16),
        "wfc1": _retile_w(wfc1_eff, MF1).astype(bf16),
        "bfc1": np.ascontiguousarray(bfc1_eff.reshape(MF1, P).T),
        "wfc2": _retile_w(f32(fc2_w), KD).astype(bf16),
        "bfc2r": np.ascontiguousarray(f32(fc2_b)[None, :]).astype(bf16),
    }
    in_maps = []
    for b in range(N_CORES):
        m = dict(shared)
        # xt[p, s, n] = x[b, n, s*P + p]
        m["xt"] = np.ascontiguousarray(
            x[b].reshape(NTOK, KD, P).transpose(2, 1, 0)
        ).astype(bf16)
        in_maps.append(m)
    return in_maps


def kernel(**inputs):
    nc = _get_nc()
    in_maps = prep_inputs(**inputs)
    res = run_bass_kernel_spmd(nc, in_maps, list(range(N_CORES)))
    outs = []
    for b in range(N_CORES):
        ytile = res.results[b]["yt"]  # [P, KD, NTOK]
        outs.append(ytile.transpose(2, 1, 0).reshape(NTOK, D))
    return np.stack(outs).astype(np.float32)
